# revision 1
# baseline (speedup 1.0000x reference)
"""CoAttenBlock Trainium2 kernel.

Full inputs in, full outputs out. Shards batch (B=8) across 8 NeuronCores,
one sample per core (pure data parallel, no collectives).

Per-core math (C=64, HW=2304, strips of 128 along the left position axis n):
  XL = WL @ [xlh;xll] + bL                      [64, 2304]
  XR = WR @ [xrh;xrl] + bR
  per strip s: aff_s = XL[:,s].T @ XR           [128, 2304]
               E_s   = exp(aff_s), rowsum via activation accum_out
               r2    = 1/rowsum  (folded into the strip's YRT weight columns)
               P12  += [YLT_s | YRT_s*r2].T @ E_s   (PSUM-resident [128, 2304])
  where YLT = (WLo_r @ XL).T strips, YRT = (WRo_r @ XR).T strips, so that
  P1 = WLo_r @ (XL @ E) and P2 = WRo_r @ (XR @ diag(r2) @ E).
  Gate pre-acts are recovered as vL.T @ P1 with vL = solve(WLo_r.T, gwL.T)
  (host-side 64x64 solve; inputs are deterministic, conditioning ~5e2).
  colsum = ones.T @ (sum of E_s)  (two SBUF accumulators: DVE + GPSIMD chains)
  s1 = sigmoid(g1pre * r1 + gb1) * r1,  r1 = 1/colsum ; s2 = sigmoid(g2pre+gb2)
  out_L = WLo_l @ XL + P1 * s1[m] + bLo ; out_R = WRo_l @ XR + P2 * s2[m] + bRo

float32r (single-pass PE mode) is used for all large matmuls; producers of
f32r-matmul inputs write with a f32r-typed output AP so the engine rounds on
write (BIR verifier requirement). Convs and YLT/YRT stay higher precision.
"""

import os
import sys

import numpy as np

if os.path.isdir("/opt/trn_rl_repo") and "/opt/trn_rl_repo" not in sys.path:
    sys.path.insert(0, "/opt/trn_rl_repo")

import concourse.bass as bass
import concourse.tile as tile
from concourse import bacc, mybir
from concourse.bass_utils import run_bass_kernel_spmd

B, C, H, W = 8, 64, 48, 48
HW = H * W            # 2304
C2 = 2 * C            # 128
NSTRIP = HW // 128    # 18
F32 = mybir.dt.float32
F32R = mybir.dt.float32r
AF = mybir.ActivationFunctionType


def chunks(total, step=512):
    out = []
    c0 = 0
    while c0 < total:
        out.append((c0, min(step, total - c0)))
        c0 += step
    return out


CH_2304 = chunks(2304)            # 4x512 + 256


def r(ap):
    return ap.bitcast(F32R)


def build_nc():
    nc = bacc.Bacc("TRN2", target_bir_lowering=False, debug=False)

    x2l_d = nc.dram_tensor("x2l", [C2, HW], F32, kind="ExternalInput").ap()
    x2r_d = nc.dram_tensor("x2r", [C2, HW], F32, kind="ExternalInput").ap()
    wlT_d = nc.dram_tensor("wlT", [C2, C], F32, kind="ExternalInput").ap()
    wrT_d = nc.dram_tensor("wrT", [C2, C], F32, kind="ExternalInput").ap()
    wloRT_d = nc.dram_tensor("wloRT", [C, C], F32, kind="ExternalInput").ap()
    wroRT_d = nc.dram_tensor("wroRT", [C, C], F32, kind="ExternalInput").ap()
    wloLT_d = nc.dram_tensor("wloLT", [C, C], F32, kind="ExternalInput").ap()
    wroLT_d = nc.dram_tensor("wroLT", [C, C], F32, kind="ExternalInput").ap()
    vlr_d = nc.dram_tensor("vlr", [C2, 1], F32, kind="ExternalInput").ap()
    bL_d = nc.dram_tensor("bL", [C, 1], F32, kind="ExternalInput").ap()
    bR_d = nc.dram_tensor("bR", [C, 1], F32, kind="ExternalInput").ap()
    bLo_d = nc.dram_tensor("bLo", [C, 1], F32, kind="ExternalInput").ap()
    bRo_d = nc.dram_tensor("bRo", [C, 1], F32, kind="ExternalInput").ap()
    gb_d = nc.dram_tensor("gb", [1, 2], F32, kind="ExternalInput").ap()
    # identity stacked twice: rows 0:64 and 64:128 both hold I_64, so id-adds
    # can source either half of a [128, ...] tile at matching base partition
    id2_np = np.vstack([np.eye(C, dtype=np.float32), np.eye(C, dtype=np.float32)])
    id64b_d = nc.inline_tensor(id2_np, "id64b").ap()
    # selector for the merged S12 broadcast: out rows 0:64 <- s1, 64:128 <- g2
    sel_np = np.zeros((2, C2), dtype=np.float32)
    sel_np[0, 0:C] = 1.0
    sel_np[1, C:C2] = 1.0
    sel12_d = nc.inline_tensor(sel_np, "sel12").ap()

    out_l_d = nc.dram_tensor("out_l", [C, HW], F32, kind="ExternalOutput").ap()
    out_r_d = nc.dram_tensor("out_r", [C, HW], F32, kind="ExternalOutput").ap()

    with tile.TileContext(nc) as tc:
        import contextlib

        with contextlib.ExitStack() as outer:
            consts = outer.enter_context(tc.tile_pool(name="consts", bufs=1))
            big = outer.enter_context(tc.tile_pool(name="big", bufs=1))
            epool = outer.enter_context(tc.tile_pool(name="epool", bufs=4))
            smalls = outer.enter_context(tc.tile_pool(name="smalls", bufs=3))
            ph3sb = outer.enter_context(tc.tile_pool(name="ph3sb", bufs=2))

            # ---- constants / weights to SBUF ----
            wlT = consts.tile([C2, C], F32)
            wrT = consts.tile([C2, C], F32)
            wloRT_raw = consts.tile([C, C], F32)
            wroRT_raw = consts.tile([C, C], F32)
            wloLT_raw = consts.tile([C, C], F32)
            wroLT_raw = consts.tile([C, C], F32)
            vlr_raw = consts.tile([C2, 1], F32)
            id64b_raw = consts.tile([C2, C], F32)
            sel12_raw = consts.tile([2, C2], F32)
            ones128_raw = consts.tile([C2, 1], F32)
            ones64_raw = consts.tile([1, C], F32)
            wloRT = consts.tile([C, C], F32)
            wroRT = consts.tile([C, C], F32)
            wloLT = consts.tile([C, C], F32)
            wroLT = consts.tile([C, C], F32)
            vlr = consts.tile([C2, 1], F32)
            id64b = consts.tile([C2, C], F32)
            sel12 = consts.tile([2, C2], F32)
            ones128 = consts.tile([C2, 1], F32)
            ones64 = consts.tile([1, C], F32)
            bL = consts.tile([C, 1], F32)
            bR = consts.tile([C, 1], F32)
            bLo = consts.tile([C, 1], F32)
            bRo = consts.tile([C, 1], F32)
            gb = consts.tile([1, 2], F32)
            for dst, src in [
                (r(wlT), r(wlT_d)), (r(wrT), r(wrT_d)),
                (wloRT_raw, wloRT_d), (wroRT_raw, wroRT_d),
                (wloLT_raw, wloLT_d), (wroLT_raw, wroLT_d),
                (vlr_raw, vlr_d), (id64b_raw, id64b_d), (sel12_raw, sel12_d),
                (bL, bL_d), (bR, bR_d), (bLo, bLo_d), (bRo, bRo_d),
                (gb, gb_d),
            ]:
                nc.sync.dma_start(out=dst, in_=src)
            nc.vector.memset(ones128_raw, 1.0)
            nc.vector.memset(ones64_raw, 1.0)
            for dst, srcc in [(ones128, ones128_raw), (ones64, ones64_raw),
                              (wloRT, wloRT_raw), (wroRT, wroRT_raw),
                              (wloLT, wloLT_raw), (wroLT, wroLT_raw),
                              (vlr, vlr_raw), (id64b, id64b_raw),
                              (sel12, sel12_raw)]:
                nc.scalar.copy(r(dst), srcc)

            # ---- big SBUF tensors ----
            x2l = big.tile([C2, HW], F32)
            x2r = big.tile([C2, HW], F32)
            XL = big.tile([C, HW], F32)
            XR = big.tile([C, HW], F32)
            Wc = big.tile([C2, HW], F32)       # 18 strips of [YLT | YRT]
            csum_a = big.tile([C2, HW // 2], F32)  # DVE accumulates m[0:1152]
            csum_b = big.tile([C2, HW // 2], F32)  # Pool accumulates m[1152:]
            P12sb = big.tile([C2, HW], F32)    # drained P1 (0:64) / P2 (64:128)
            outLR = big.tile([C2, HW], F32)

            for c0, cn in CH_2304:
                nc.sync.dma_start(out=r(x2l[:, c0:c0 + cn]),
                                  in_=r(x2l_d[:, c0:c0 + cn]))
                nc.sync.dma_start(out=r(x2r[:, c0:c0 + cn]),
                                  in_=r(x2r_d[:, c0:c0 + cn]))

            with contextlib.ExitStack() as ph2_psum:
                p12p = ph2_psum.enter_context(
                    tc.tile_pool(name="p12p", bufs=1, space="PSUM"))
                affp = ph2_psum.enter_context(
                    tc.tile_pool(name="affp", bufs=1, space="PSUM"))
                P12 = p12p.tile([C2, HW], F32)  # 5 banks, lives all of phase 1+2
                ring = affp.tile([C2, 1536], F32, tag="ring", name="aff_ring")

                # ---- phase 1: convs (full fp32) + YLT/YRT build ----
                # After conv chunk j, emit the YLT/YRT strips of chunk j-1
                # (their XL/XR columns are copied by then); P12 is scratch.
                def emit_y(t):
                    ysl = slice(64 * t, 64 * t + 64)
                    nc.tensor.matmul(P12[:, ysl],
                                     r(XL[:, 128 * t:128 * t + 128]),
                                     r(wloRT), start=True, stop=True)
                    nc.vector.tensor_copy(r(Wc[:, 128 * t:128 * t + 64]),
                                          P12[:, ysl])
                    ysr = slice(64 * (NSTRIP + t), 64 * (NSTRIP + t) + 64)
                    nc.tensor.matmul(P12[:, ysr],
                                     r(XR[:, 128 * t:128 * t + 128]),
                                     r(wroRT), start=True, stop=True)
                    nc.vector.tensor_copy(r(Wc[:, 128 * t + 64:128 * t + 128]),
                                          P12[:, ysr])

                for j, (c0, cn) in enumerate(CH_2304):
                    nc.tensor.matmul(P12[0:C, c0:c0 + cn], r(wlT),
                                     r(x2l[:, c0:c0 + cn]), start=True, stop=True)
                    nc.scalar.activation(r(XL[:, c0:c0 + cn]),
                                         P12[0:C, c0:c0 + cn],
                                         AF.Identity, bias=bL, scale=1.0)
                    rsl = (j % 3) * 512
                    nc.tensor.matmul(ring[0:C, rsl:rsl + cn], r(wrT),
                                     r(x2r[:, c0:c0 + cn]), start=True, stop=True)
                    nc.vector.tensor_scalar_add(r(XR[:, c0:c0 + cn]),
                                                ring[0:C, rsl:rsl + cn], bR)
                    if j > 0:
                        for t in range(4 * (j - 1), 4 * j):
                            emit_y(t)
                for t in range(4 * (len(CH_2304) - 1), NSTRIP):
                    emit_y(t)

                # ---- phase 2: strip loop over a 3-slot aff ring ----
                # A_s = aff matmuls + merged exps + rowsum/recip for strip s.
                # B_s = Wc scale + bacc matmuls + colsum accumulate for s.
                # B lags A by 2 strips so PE always has bacc work to fill exp
                # waits; the YLT/YRT -> Wc build is emitted during the lag.
                phase = 0
                r2s = {}

                def emit_bacc(sb, c0, cn):
                    nc.tensor.matmul(P12[:, c0:c0 + cn],
                                     r(Wc[:, 128 * sb:128 * sb + 128]),
                                     r(Es[sb][:, c0:c0 + cn]),
                                     start=(sb == 0), stop=(sb == NSTRIP - 1))

                def emit_csum(sb):
                    E = Es[sb]
                    half = HW // 2
                    if sb == 0:
                        nc.vector.tensor_copy(r(csum_a), E[:, 0:half])
                        nc.gpsimd.tensor_copy(r(csum_b), E[:, half:HW])
                    else:
                        nc.vector.tensor_add(r(csum_a), csum_a, E[:, 0:half])
                        nc.gpsimd.tensor_add(r(csum_b), csum_b, E[:, half:HW])

                def emit_A(s, phase, sb):
                    # aff+exp for strip s, with strip sb's bacc matmuls
                    # interleaved between the aff pieces (PE is in-order; this
                    # keeps ACT fed with the next exp as early as possible).
                    if sb >= 0:
                        wright = Wc[:, 128 * sb + 64:128 * sb + 128]
                        nc.vector.tensor_scalar_mul(r(wright), wright, r2s[sb])
                    E = epool.tile([C2, HW], F32, tag="e", name=f"E_{s}")
                    rs = smalls.tile([C2, 4], F32, tag="rs", name=f"rs_{s}")
                    lhs_aff = r(XL[:, 128 * s:128 * s + 128])
                    pieces = [(p0, pn, (phase + i) % 3)
                              for i, (p0, pn) in enumerate(CH_2304)]
                    groups = []
                    for p0, pn, sl in pieces:
                        if groups and groups[-1][2] + groups[-1][1] == sl * 512 \
                                and groups[-1][1] + pn <= 1536:
                            groups[-1][1] += pn
                        else:
                            groups.append([p0, pn, sl * 512])
                    gidx = 0
                    done = 0
                    for i, (p0, pn, sl) in enumerate(pieces):
                        nc.tensor.matmul(ring[:, sl * 512:sl * 512 + pn],
                                         lhs_aff, r(XR[:, p0:p0 + pn]),
                                         start=True, stop=True)
                        done += pn
                        while gidx < len(groups) and \
                                groups[gidx][0] + groups[gidx][1] <= done:
                            m0, mn, r0 = groups[gidx]
                            nc.scalar.activation(r(E[:, m0:m0 + mn]),
                                                 ring[:, r0:r0 + mn], AF.Exp,
                                                 accum_out=rs[:, gidx:gidx + 1])
                            gidx += 1
                        if sb >= 0 and i < len(CH_2304):
                            bc0, bcn = CH_2304[i]
                            emit_bacc(sb, bc0, bcn)
                    rowsum = smalls.tile([C2, 1], F32, tag="rowsum",
                                         name=f"rowsum_{s}")
                    r2 = smalls.tile([C2, 1], F32, tag="r2", name=f"r2_{s}",
                                     bufs=4)
                    nc.vector.tensor_reduce(rowsum, rs[:, 0:len(groups)],
                                            axis=mybir.AxisListType.X,
                                            op=mybir.AluOpType.add)
                    nc.vector.reciprocal(r2, rowsum)
                    r2s[s] = r2
                    if sb >= 0:
                        emit_csum(sb)
                    return E

                def emit_B_tail(sb):
                    wright = Wc[:, 128 * sb + 64:128 * sb + 128]
                    nc.vector.tensor_scalar_mul(r(wright), wright, r2s[sb])
                    for c0, cn in CH_2304:
                        emit_bacc(sb, c0, cn)
                    emit_csum(sb)

                Es = {}
                Es = {}

                for s in range(NSTRIP):
                    Es[s] = emit_A(s, phase, s - 2)
                    phase = (phase + len(CH_2304)) % 3
                for s in (NSTRIP - 2, NSTRIP - 1):
                    emit_B_tail(s)

                # drain P12 (both engines in parallel)
                nc.vector.tensor_copy(r(P12sb[0:C, :]), P12[0:C, :])
                nc.scalar.copy(r(P12sb[C:C2, :]), P12[C:C2, :])

            # ---- phase 3: 512-col pieces, one PSUM bank per role ----
            with tc.tile_pool(name="ph3p", bufs=1, space="PSUM") as ph3:
                for q, (p0, pn) in enumerate(CH_2304):
                    sl = slice(p0, p0 + pn)

                    cs = ph3.tile([1, pn], F32, tag="cs", name=f"cs_{q}",
                                  padded_shape=[1, 512])
                    half = HW // 2
                    if p0 + pn <= half:
                        nc.tensor.matmul(cs, r(ones128),
                                         r(csum_a[:, p0:p0 + pn]),
                                         start=True, stop=True)
                    elif p0 >= half:
                        nc.tensor.matmul(cs, r(ones128),
                                         r(csum_b[:, p0 - half:p0 - half + pn]),
                                         start=True, stop=True)
                    else:
                        ca = half - p0
                        nc.tensor.matmul(cs[:, 0:ca], r(ones128),
                                         r(csum_a[:, p0:half]),
                                         start=True, stop=True)
                        nc.tensor.matmul(cs[:, ca:pn], r(ones128),
                                         r(csum_b[:, 0:p0 + pn - half]),
                                         start=True, stop=True)
                    r1 = ph3sb.tile([1, pn], F32, tag="r1", name=f"r1_{q}",
                                    padded_shape=[1, 512])
                    nc.vector.reciprocal(r1, cs)

                    g1p = ph3.tile([1, pn], F32, tag="g1p", name=f"g1p_{q}",
                                   padded_shape=[1, 512])
                    nc.tensor.matmul(g1p, r(vlr[0:C]), r(P12sb[0:C, sl]),
                                     start=True, stop=True)
                    g2p = ph3.tile([1, pn], F32, tag="g2p", name=f"g2p_{q}",
                                   padded_shape=[1, 512])
                    nc.tensor.matmul(g2p, r(vlr[C:C2]), r(P12sb[C:C2, sl]),
                                     start=True, stop=True)

                    g1pre = ph3sb.tile([1, pn], F32, tag="g1pre",
                                       name=f"g1pre_{q}", padded_shape=[1, 512])
                    nc.vector.tensor_mul(g1pre, g1p, r1)
                    g1 = ph3sb.tile([1, pn], F32, tag="g1", name=f"g1_{q}",
                                    padded_shape=[1, 512])
                    nc.scalar.activation(g1, g1pre, AF.Sigmoid,
                                         bias=gb[0:1, 0:1], scale=1.0)
                    s1 = ph3sb.tile([1, pn], F32, tag="s1", name=f"s1_{q}",
                                    padded_shape=[1, 512])
                    nc.vector.tensor_mul(r(s1), g1, r1)
                    g2 = ph3sb.tile([1, pn], F32, tag="g2", name=f"g2_{q}",
                                    padded_shape=[1, 512])
                    nc.scalar.activation(r(g2), g2p, AF.Sigmoid,
                                         bias=gb[0:1, 1:2], scale=1.0)

                    S1 = ph3.tile([C, pn], F32, tag="S1", name=f"S1_{q}",
                                  padded_shape=[C, 512])
                    nc.tensor.matmul(S1, r(ones64), r(s1), start=True, stop=True)
                    S2 = ph3.tile([C, pn], F32, tag="S2", name=f"S2_{q}",
                                  padded_shape=[C, 512])
                    nc.tensor.matmul(S2, r(ones64), r(g2), start=True, stop=True)
                    t1 = ph3sb.tile([C, pn], F32, tag="t1", name=f"t1_{q}",
                                    padded_shape=[C, 512])
                    nc.vector.tensor_mul(r(t1), P12sb[0:C, sl], S1)
                    t2 = ph3sb.tile([C, pn], F32, tag="t2", name=f"t2_{q}",
                                    padded_shape=[C, 512])
                    nc.vector.tensor_mul(r(t2), P12sb[C:C2, sl], S2)

                    OL = ph3.tile([C, pn], F32, tag="OL", name=f"OL_{q}",
                                  padded_shape=[C, 512])
                    nc.tensor.matmul(OL, r(wloLT), r(XL[:, sl]),
                                     start=True, stop=False)
                    nc.tensor.matmul(OL, r(id64b[0:C]), r(t1),
                                     start=False, stop=True)
                    nc.scalar.activation(outLR[0:C, sl], OL, AF.Identity,
                                         bias=bLo, scale=1.0)
                    OR_ = ph3.tile([C, pn], F32, tag="OR", name=f"OR_{q}",
                                   padded_shape=[C, 512])
                    nc.tensor.matmul(OR_, r(wroLT), r(XR[:, sl]),
                                     start=True, stop=False)
                    nc.tensor.matmul(OR_, r(id64b[0:C]), r(t2),
                                     start=False, stop=True)
                    nc.scalar.activation(outLR[C:C2, sl], OR_, AF.Identity,
                                         bias=bRo, scale=1.0)
                    nc.sync.dma_start(out=out_l_d[:, sl], in_=outLR[0:C, sl])
                    nc.sync.dma_start(out=out_r_d[:, sl], in_=outLR[C:C2, sl])

    nc.compile()
    return nc


_NC_CACHE = {}


def _get_nc():
    if "nc" not in _NC_CACHE:
        _NC_CACHE["nc"] = build_nc()
    return _NC_CACHE["nc"]


def _prep_shared(concaL_w, concaL_b, concaR_w, concaR_b,
                 gateL_w, gateL_b, gateR_w, gateR_b,
                 concaLo_w, concaLo_b, concaRo_w, concaRo_b):
    f = np.float32
    wloR = np.asarray(concaLo_w)[:, C:].astype(np.float64)
    wroR = np.asarray(concaRo_w)[:, C:].astype(np.float64)
    vL = np.linalg.solve(wloR.T, np.asarray(gateL_w).astype(np.float64).reshape(C))
    vR = np.linalg.solve(wroR.T, np.asarray(gateR_w).astype(np.float64).reshape(C))
    vlr = np.concatenate([vL, vR]).reshape(C2, 1)
    return {
        "wlT": np.ascontiguousarray(np.asarray(concaL_w).T, dtype=f),
        "wrT": np.ascontiguousarray(np.asarray(concaR_w).T, dtype=f),
        "wloRT": np.ascontiguousarray(wloR.T, dtype=f),
        "wroRT": np.ascontiguousarray(wroR.T, dtype=f),
        "wloLT": np.ascontiguousarray(np.asarray(concaLo_w)[:, :C].T, dtype=f),
        "wroLT": np.ascontiguousarray(np.asarray(concaRo_w)[:, :C].T, dtype=f),
        "vlr": np.ascontiguousarray(vlr, dtype=f),
        "bL": np.ascontiguousarray(np.asarray(concaL_b).reshape(C, 1), dtype=f),
        "bR": np.ascontiguousarray(np.asarray(concaR_b).reshape(C, 1), dtype=f),
        "bLo": np.ascontiguousarray(np.asarray(concaLo_b).reshape(C, 1), dtype=f),
        "bRo": np.ascontiguousarray(np.asarray(concaRo_b).reshape(C, 1), dtype=f),
        "gb": np.array([[np.asarray(gateL_b).reshape(()),
                         np.asarray(gateR_b).reshape(())]], dtype=f),
    }


def kernel(xlh, xll, xrh, xrl,
           concaL_w, concaL_b, concaR_w, concaR_b,
           gateL_w, gateL_b, gateR_w, gateR_b,
           concaLo_w, concaLo_b, concaRo_w, concaRo_b,
           _return_results=False):
    nc = _get_nc()
    shared = _prep_shared(concaL_w, concaL_b, concaR_w, concaR_b,
                          gateL_w, gateL_b, gateR_w, gateR_b,
                          concaLo_w, concaLo_b, concaRo_w, concaRo_b)
    xlh = np.asarray(xlh, dtype=np.float32)
    xll = np.asarray(xll, dtype=np.float32)
    xrh = np.asarray(xrh, dtype=np.float32)
    xrl = np.asarray(xrl, dtype=np.float32)

    in_maps = []
    for c in range(B):
        x2l = np.concatenate([xlh[c].reshape(C, HW), xll[c].reshape(C, HW)], axis=0)
        x2r = np.concatenate([xrh[c].reshape(C, HW), xrl[c].reshape(C, HW)], axis=0)
        m = dict(shared)
        m["x2l"] = np.ascontiguousarray(x2l)
        m["x2r"] = np.ascontiguousarray(x2r)
        in_maps.append(m)

    # The first execution of a freshly compiled NEFF occasionally hits a
    # transient NRT_EXEC_UNIT_UNRECOVERABLE on this axon setup; an immediate
    # re-dispatch of the same executable has always succeeded, so retry.
    res = None
    for attempt in range(3):
        try:
            res = run_bass_kernel_spmd(nc, in_maps, list(range(B)))
            break
        except Exception:
            if attempt == 2:
                raise
            import time as _time
            _time.sleep(2.0)
    out_L = np.stack([res.results[c]["out_l"].reshape(C, H, W) for c in range(B)])
    out_R = np.stack([res.results[c]["out_r"].reshape(C, H, W) for c in range(B)])
    if _return_results:
        return (out_L, out_R), res
    return (out_L, out_R)



# revision 16
# speedup vs baseline: 1.1234x; 1.1234x over previous
"""CoAttenBlock Trainium2 kernel.

Full inputs in, full outputs out. Shards batch (B=8) across 8 NeuronCores,
one sample per core (pure data parallel, no collectives).

Per-core math (C=64, HW=2304, strips of 128 along the left position axis n):
  XL = WL @ [xlh;xll] + bL                      [64, 2304]
  XR = WR @ [xrh;xrl] + bR
  per strip s: aff_s = XL[:,s].T @ XR           [128, 2304]
               E_s   = exp(aff_s), rowsum via activation accum_out
               r2    = 1/rowsum  (folded into the strip's YRT weight columns)
               P12  += [YLT_s | YRT_s*r2].T @ E_s   (PSUM-resident [128, 2304])
  where YLT = (WLo_r @ XL).T strips, YRT = (WRo_r @ XR).T strips, so that
  P1 = WLo_r @ (XL @ E) and P2 = WRo_r @ (XR @ diag(r2) @ E).
  Gate pre-acts are recovered as vL.T @ P1 with vL = solve(WLo_r.T, gwL.T)
  (host-side 64x64 solve; inputs are deterministic, conditioning ~5e2).
  colsum accumulates on DVE (cols 0:1536) + GPSIMD (1536:2304).
  Tail gate math runs in a transposed [128,18] layout (column m = 128*mt+mp
  lives at partition mp, col mt) so ACT/DVE ops are 18 cols wide, then a
  bf16 PE transpose + SBUF->SBUF DMA rebuilds the row form for the
  ones-broadcast matmuls:
  s1 = sigmoid(g1pre*r1 + gb1)*r1, r1 = 1/colsum ; s2 = sigmoid(g2pre+gb2)
  out_L = WLo_l @ XLa + P1*s1[m] ; out_R = WRo_l @ XRa + P2*s2[m]
  (bLo/bRo folded in via an augmented ones row 64 on XLa/XRa).

float32r (single-pass PE mode) is used for all large matmuls; producers of
f32r-matmul inputs write with a f32r-typed output AP so the engine rounds on
write (BIR verifier requirement).
"""

import os
import sys

import numpy as np

if os.path.isdir("/opt/trn_rl_repo") and "/opt/trn_rl_repo" not in sys.path:
    sys.path.insert(0, "/opt/trn_rl_repo")

import concourse.bass as bass
import concourse.tile as tile
from concourse import bacc, mybir
from concourse.bass_utils import run_bass_kernel_spmd

B, C, H, W = 8, 64, 48, 48
HW = H * W            # 2304
C2 = 2 * C            # 128
NSTRIP = HW // 128    # 18
F32 = mybir.dt.float32
F32R = mybir.dt.float32r
BF16 = mybir.dt.bfloat16
AF = mybir.ActivationFunctionType

CSPLIT = 1408         # csum: DVE cols [0:CSPLIT], Pool cols [CSPLIT:HW]


def chunks(total, step=512):
    out = []
    c0 = 0
    while c0 < total:
        out.append((c0, min(step, total - c0)))
        c0 += step
    return out


CH_2304 = chunks(2304)            # 4x512 + 256
DMA_CH = chunks(2304, 768)        # 3x768 input DMA chunks

# f32 weight-pack column layout
PK_WLT = slice(0, 64)
PK_WRT = slice(64, 128)
PK_WLORT = slice(128, 192)
PK_WRORT = slice(192, 256)
PK_WLOLTA = slice(256, 320)
PK_WROLTA = slice(320, 384)
PK_VLR2 = slice(384, 386)
PK_GBL = slice(386, 387)
PK_GBR = slice(387, 388)
PK_BLROW = slice(388, 452)
PK_BRROW = slice(452, 516)
PK_ID64 = slice(516, 580)
PK_BRCOL = slice(580, 581)
PK_ONECOL = slice(581, 582)
PKF_COLS = 582
PKB_COLS = 192                    # bf16 pack: id128 [0:128], ones64 row [128:192]


def r(ap):
    return ap.bitcast(F32R)


def build_nc():
    nc = bacc.Bacc("TRN2", target_bir_lowering=False, debug=False)

    x2l_d = nc.dram_tensor("x2l", [C2, HW], F32, kind="ExternalInput").ap()
    x2r_d = nc.dram_tensor("x2r", [C2, HW], F32, kind="ExternalInput").ap()
    pkf_d = nc.dram_tensor("pkf", [C2, PKF_COLS], F32, kind="ExternalInput").ap()
    pkb_d = nc.dram_tensor("pkb", [C2, PKB_COLS], BF16, kind="ExternalInput").ap()
    ones_np = np.ones((1, HW), dtype=np.float32)
    ones_d = nc.inline_tensor(ones_np, "onesrow").ap()

    out_l_d = nc.dram_tensor("out_l", [C, HW], F32, kind="ExternalOutput").ap()
    out_r_d = nc.dram_tensor("out_r", [C, HW], F32, kind="ExternalOutput").ap()

    with tile.TileContext(nc) as tc:
        import contextlib

        with contextlib.ExitStack() as outer:
            consts = outer.enter_context(tc.tile_pool(name="consts", bufs=1))
            big = outer.enter_context(tc.tile_pool(name="big", bufs=1))
            epool = outer.enter_context(tc.tile_pool(name="epool", bufs=5))
            smalls = outer.enter_context(tc.tile_pool(name="smalls", bufs=3))
            tailsb = outer.enter_context(tc.tile_pool(name="tailsb", bufs=1))

            pk = consts.tile([C2, PKF_COLS], F32)
            pkb = consts.tile([C2, PKB_COLS], BF16)

            # ---- big SBUF tensors ----
            x2l = big.tile([C2, HW], F32)
            x2r = big.tile([C2, HW], F32)
            XLa = big.tile([C + 1, HW], F32)   # row 64 = ones (bias fold)
            XRa = big.tile([C + 1, HW], F32)
            Wc = big.tile([C2, HW], F32)       # 18 strips of [YLT | YRT]
            csum_a = big.tile([C2, CSPLIT], F32)       # DVE accumulator
            csum_b = big.tile([C2, HW - CSPLIT], F32)  # Pool accumulator
            P12sb = big.tile([C2, HW], F32)    # drained P1 (0:64) / P2 (64:128)
            outLR = big.tile([C2, HW], F32)

            # DMA order: weights, xl chunk0, xr all, xl rest. First conv can
            # start ~2.7us in; XR (needed for every aff piece) lands early.
            nc.sync.dma_start(out=r(pk), in_=r(pkf_d))
            nc.sync.dma_start(out=pkb, in_=pkb_d)
            (l0, ln) = DMA_CH[0]
            nc.sync.dma_start(out=r(x2l[:, l0:l0 + ln]), in_=r(x2l_d[:, l0:l0 + ln]))
            # ones rows for bias folds (rank-1 rhs and aug-conv row 64)
            nc.sync.dma_start(out=r(XLa[C:C + 1, :]), in_=r(ones_d))
            nc.sync.dma_start(out=r(XRa[C:C + 1, :]), in_=r(ones_d))
            for c0, cn in DMA_CH:
                nc.sync.dma_start(out=r(x2r[:, c0:c0 + cn]),
                                  in_=r(x2r_d[:, c0:c0 + cn]))
            for c0, cn in DMA_CH[1:]:
                nc.sync.dma_start(out=r(x2l[:, c0:c0 + cn]),
                                  in_=r(x2l_d[:, c0:c0 + cn]))

            wlT = r(pk[:, PK_WLT])
            wrT = r(pk[:, PK_WRT])
            wloRT = r(pk[0:C, PK_WLORT])
            wroRT = r(pk[0:C, PK_WRORT])
            wloLTa = r(pk[0:C + 1, PK_WLOLTA])
            wroLTa = r(pk[0:C + 1, PK_WROLTA])
            vlr2f = pk[:, PK_VLR2]
            gbL = pk[:, PK_GBL]
            gbR = pk[:, PK_GBR]
            bLrow = r(pk[C:C + 1, PK_BLROW])
            bRcol = pk[0:C, PK_BRCOL]
            id64 = r(pk[0:C, PK_ID64])
            onecol = pk[:, PK_ONECOL]
            id128b = pkb[:, 0:128]
            ones64b = pkb[:, 128:192]

            with contextlib.ExitStack() as ph2_psum:
                p12p = ph2_psum.enter_context(
                    tc.tile_pool(name="p12p", bufs=1, space="PSUM"))
                affp = ph2_psum.enter_context(
                    tc.tile_pool(name="affp", bufs=1, space="PSUM"))
                P12 = p12p.tile([C2, HW], F32)  # lives all of phase 1+2
                ring = affp.tile([C2, 1536], F32, tag="ring", name="aff_ring")

                # ---- phase 1: convs (PSUM scratch: P12 for L, ring for R),
                # bias via rank-1 matmul, drains split DVE / ACT+Pool,
                # interleaved with strip-0 affs so exp starts ASAP ----
                def conv_piece(j):
                    c0, cn = CH_2304[j]
                    sl = slice(c0, c0 + cn)
                    nc.tensor.matmul(P12[0:C, sl], wlT, r(x2l[:, sl]),
                                     start=True, stop=False)
                    nc.tensor.matmul(P12[0:C, sl], bLrow, r(XLa[C:C + 1, sl]),
                                     start=False, stop=True)
                    nc.vector.tensor_copy(r(XLa[0:C, sl]), P12[0:C, sl])
                    rsl = (j % 3) * 512
                    nc.tensor.matmul(ring[0:C, rsl:rsl + cn], wrT, r(x2r[:, sl]),
                                     start=True, stop=True)
                    nc.scalar.activation(r(XRa[0:C, sl]), ring[0:C, rsl:rsl + cn],
                                         AF.Identity, bias=bRcol, scale=1.0)

                # ---- phase 2 helpers ----
                def emit_y(t, drain_eng):
                    # Wc strip t = [YLT_t | YRT_t]; PSUM scratch inside P12
                    # (pre-bacc window only: strips 0-1 of the loop).
                    ysl = slice(128 * t, 128 * t + 64)
                    nc.tensor.matmul(P12[:, ysl],
                                     r(XLa[0:C, 128 * t:128 * t + 128]),
                                     wloRT, start=True, stop=True)
                    ysr = slice(128 * t + 64, 128 * t + 128)
                    nc.tensor.matmul(P12[:, ysr],
                                     r(XRa[0:C, 128 * t:128 * t + 128]),
                                     wroRT, start=True, stop=True)
                    dst = r(Wc[:, 128 * t:128 * t + 128])
                    srcp = P12[:, 128 * t:128 * t + 128]
                    if drain_eng == "dve":
                        nc.vector.tensor_copy(dst, srcp)
                    else:
                        nc.scalar.copy(dst, srcp)

                phase = 0
                r2s = {}

                def emit_bacc(sb, c0, cn):
                    nc.tensor.matmul(P12[:, c0:c0 + cn],
                                     r(Wc[:, 128 * sb:128 * sb + 128]),
                                     r(Es[sb][:, c0:c0 + cn]),
                                     start=(sb == 0), stop=(sb == NSTRIP - 1))

                def emit_csum(sb):
                    E = Es[sb]
                    if sb == 0:
                        nc.vector.tensor_copy(r(csum_a), E[:, 0:CSPLIT])
                        nc.gpsimd.tensor_copy(r(csum_b), E[:, CSPLIT:HW])
                    else:
                        nc.vector.tensor_add(r(csum_a), csum_a, E[:, 0:CSPLIT])
                        nc.gpsimd.tensor_add(r(csum_b), csum_b, E[:, CSPLIT:HW])

                def emit_A(s, phase, sb):
                    # aff+exp for strip s, with strip sb's bacc matmuls
                    # interleaved between the aff pieces; strips 0-1 instead
                    # interleave conv pieces + the 36 emit_y builds.
                    if sb >= 0:
                        wright = Wc[:, 128 * sb + 64:128 * sb + 128]
                        nc.vector.tensor_scalar_mul(r(wright), wright, r2s[sb])
                    E = epool.tile([C2, HW], F32, tag="e", name=f"E_{s}")
                    rs = smalls.tile([C2, 4], F32, tag="rs", name=f"rs_{s}")
                    lhs_aff = r(XLa[0:C, 128 * s:128 * s + 128])
                    pieces = [(p0, pn, (phase + i) % 3)
                              for i, (p0, pn) in enumerate(CH_2304)]
                    groups = []
                    for p0, pn, sl in pieces:
                        if groups and groups[-1][2] + groups[-1][1] == sl * 512 \
                                and groups[-1][1] + pn <= 1536:
                            groups[-1][1] += pn
                        else:
                            groups.append([p0, pn, sl * 512])
                    gidx = 0
                    done = 0
                    for i, (p0, pn, sl) in enumerate(pieces):
                        if s == 0 and i < len(CH_2304):
                            conv_piece(i)
                        nc.tensor.matmul(ring[:, sl * 512:sl * 512 + pn],
                                         lhs_aff, r(XRa[0:C, p0:p0 + pn]),
                                         start=True, stop=True)
                        done += pn
                        while gidx < len(groups) and \
                                groups[gidx][0] + groups[gidx][1] <= done:
                            m0, mn, r0 = groups[gidx]
                            nc.scalar.activation(r(E[:, m0:m0 + mn]),
                                                 ring[:, r0:r0 + mn], AF.Exp,
                                                 accum_out=rs[:, gidx:gidx + 1])
                            gidx += 1
                        if sb >= 0 and i < len(CH_2304):
                            bc0, bcn = CH_2304[i]
                            emit_bacc(sb, bc0, bcn)
                        if s in (0, 1):
                            base = 0 if s == 0 else 10
                            for t in range(base + 2 * i, base + 2 * i + 2):
                                if t < NSTRIP:
                                    emit_y(t, "act" if (t % 6 == 0) else "dve")
                    rowsum = smalls.tile([C2, 1], F32, tag="rowsum",
                                         name=f"rowsum_{s}")
                    r2 = smalls.tile([C2, 1], F32, tag="r2", name=f"r2_{s}",
                                     bufs=4)
                    nc.vector.tensor_reduce(rowsum, rs[:, 0:len(groups)],
                                            axis=mybir.AxisListType.X,
                                            op=mybir.AluOpType.add)
                    nc.vector.reciprocal(r2, rowsum)
                    r2s[s] = r2
                    if sb >= 0:
                        emit_csum(sb)
                    return E

                def emit_B_tail(sb):
                    wright = Wc[:, 128 * sb + 64:128 * sb + 128]
                    nc.vector.tensor_scalar_mul(r(wright), wright, r2s[sb])
                    # csum first: the tail gate chain waits on the full colsum,
                    # so get it onto DVE/Pool before the P12 drains queue up.
                    emit_csum(sb)
                    drain_eng = ["dve", "act", "dve", "act", "dve"]
                    for j, (c0, cn) in enumerate(CH_2304):
                        emit_bacc(sb, c0, cn)
                        if sb == NSTRIP - 1:
                            # P12 piece is final once the last strip's bacc
                            # for it retires: drain immediately.
                            sl = slice(c0, c0 + cn)
                            if drain_eng[j] == "act":
                                nc.scalar.copy(r(P12sb[:, sl]), P12[:, sl])
                            elif drain_eng[j] == "dve":
                                nc.vector.tensor_copy(r(P12sb[:, sl]), P12[:, sl])
                            else:
                                nc.gpsimd.tensor_copy(r(P12sb[:, sl]), P12[:, sl])

                Es = {}
                for s in range(NSTRIP):
                    Es[s] = emit_A(s, phase, s - 2)
                    phase = (phase + len(CH_2304)) % 3
                for s in (NSTRIP - 2, NSTRIP - 1):
                    emit_B_tail(s)

            # ---- phase 3: transposed gate chain + broadcast/gate/out ----
            with tc.tile_pool(name="ph3p", bufs=1, space="PSUM") as ph3:
                cst = ph3.tile([C2, NSTRIP], F32, padded_shape=[C2, 512])
                gpt = ph3.tile([C2, 2 * NSTRIP], F32, padded_shape=[C2, 512])
                # colsum transposed: cst[mp, mt] = sum_p csum[p, 128*mt+mp]
                for mt in range(NSTRIP):
                    m0 = 128 * mt
                    src = csum_a if m0 < CSPLIT else csum_b
                    off = m0 if m0 < CSPLIT else m0 - CSPLIT
                    nc.tensor.matmul(cst[:, mt:mt + 1],
                                     src[:, off:off + 128], onecol,
                                     start=True, stop=True)
                    # gate pre-acts transposed, both sides at once
                    nc.tensor.matmul(gpt[:, 2 * mt:2 * mt + 2],
                                     P12sb[:, m0:m0 + 128], vlr2f,
                                     start=True, stop=True)
                r1t = tailsb.tile([C2, NSTRIP], F32)
                nc.vector.reciprocal(r1t, cst)
                g1pre = tailsb.tile([C2, NSTRIP], F32)
                nc.vector.tensor_mul(g1pre, gpt[:, 0::2], r1t)
                g1t = tailsb.tile([C2, NSTRIP], F32)
                s12j = tailsb.tile([C2, 2 * NSTRIP], BF16)
                nc.scalar.activation(g1t, g1pre, AF.Sigmoid, bias=gbL, scale=1.0)
                nc.scalar.activation(s12j[:, NSTRIP:], gpt[:, 1::2], AF.Sigmoid,
                                     bias=gbR, scale=1.0)
                nc.vector.tensor_mul(s12j[:, 0:NSTRIP], g1t, r1t)
                # transpose to rows, rebuild [2, 2304] row layout via DMA
                s12T = ph3.tile([2 * NSTRIP, 128], BF16,
                                padded_shape=[2 * NSTRIP, 256])
                nc.tensor.transpose(s12T, s12j, id128b)
                s12Ts = tailsb.tile([2 * NSTRIP, 128], BF16)
                nc.vector.tensor_copy(s12Ts, s12T)
                srow = tailsb.tile([C + 1, HW], BF16)  # s1 row 0, s2 row 64
                nc.sync.dma_start(out=srow[0:1, :], in_=s12Ts[0:NSTRIP, :])
                nc.sync.dma_start(out=srow[C:C + 1, :], in_=s12Ts[NSTRIP:, :])

                # per-chunk: S broadcast, gate-mul, out conv + id-add, drain
                drains = ["act"] * 10
                tmuls = ["dve"] * 10
                for q, (p0, pn) in enumerate(CH_2304):
                    sl = slice(p0, p0 + pn)
                    for side in (0, 1):
                        k = 2 * q + side
                        rows = slice(0, C) if side == 0 else slice(C, C2)
                        S = ph3.tile([C, pn], F32, tag=f"S{side}",
                                     name=f"S{side}_{q}", padded_shape=[C, 512])
                        srow_r = srow[0:1, sl] if side == 0 else srow[C:C + 1, sl]
                        ones_r = ones64b[0:1, :] if side == 0 else ones64b[C:C + 1, :]
                        nc.tensor.matmul(S, ones_r, srow_r,
                                         start=True, stop=True)
                        t_ = tailsb.tile([C, pn], F32, tag=f"t{side}",
                                         name=f"t{side}_{q}",
                                         padded_shape=[C, 512], bufs=2)
                        if tmuls[k] == "dve":
                            nc.vector.tensor_mul(r(t_), P12sb[rows, sl], S)
                        else:
                            nc.gpsimd.tensor_mul(r(t_), P12sb[rows, sl], S)
                        O = ph3.tile([C, pn], F32, tag=f"O{side}",
                                     name=f"O{side}_{q}", padded_shape=[C, 512])
                        Xa = XLa if side == 0 else XRa
                        wA = wloLTa if side == 0 else wroLTa
                        nc.tensor.matmul(O, wA, r(Xa[:, sl]),
                                         start=True, stop=False)
                        nc.tensor.matmul(O, id64, r(t_),
                                         start=False, stop=True)
                        dst = outLR[rows, sl]
                        if drains[k] == "act":
                            nc.scalar.copy(r(dst), O)
                        elif drains[k] == "dve":
                            nc.vector.tensor_copy(r(dst), O)
                        else:
                            nc.gpsimd.tensor_copy(r(dst), O)
                    if p0 + pn == 1024:
                        nc.sync.dma_start(out=out_l_d[:, 0:1024],
                                          in_=outLR[0:C, 0:1024])
                        nc.sync.dma_start(out=out_r_d[:, 0:1024],
                                          in_=outLR[C:C2, 0:1024])
                nc.sync.dma_start(out=out_l_d[:, 1024:], in_=outLR[0:C, 1024:])
                nc.sync.dma_start(out=out_r_d[:, 1024:], in_=outLR[C:C2, 1024:])

    nc.compile()
    return nc


_NC_CACHE = {}


def _get_nc():
    if "nc" not in _NC_CACHE:
        _NC_CACHE["nc"] = build_nc()
    return _NC_CACHE["nc"]


def _prep_shared(concaL_w, concaL_b, concaR_w, concaR_b,
                 gateL_w, gateL_b, gateR_w, gateR_b,
                 concaLo_w, concaLo_b, concaRo_w, concaRo_b):
    f = np.float32
    wloR = np.asarray(concaLo_w)[:, C:].astype(np.float64)
    wroR = np.asarray(concaRo_w)[:, C:].astype(np.float64)
    vL = np.linalg.solve(wloR.T, np.asarray(gateL_w).astype(np.float64).reshape(C))
    vR = np.linalg.solve(wroR.T, np.asarray(gateR_w).astype(np.float64).reshape(C))

    pkf = np.zeros((C2, PKF_COLS), dtype=f)
    pkf[:, PK_WLT] = np.asarray(concaL_w).T
    pkf[:, PK_WRT] = np.asarray(concaR_w).T
    pkf[0:C, PK_WLORT] = wloR.T
    pkf[0:C, PK_WRORT] = wroR.T
    pkf[0:C, PK_WLOLTA] = np.asarray(concaLo_w)[:, :C].T
    pkf[C, PK_WLOLTA] = np.asarray(concaLo_b).reshape(C)
    pkf[0:C, PK_WROLTA] = np.asarray(concaRo_w)[:, :C].T
    pkf[C, PK_WROLTA] = np.asarray(concaRo_b).reshape(C)
    pkf[0:C, 384] = vL
    pkf[C:C2, 385] = vR
    pkf[:, 386] = np.asarray(gateL_b).reshape(())
    pkf[:, 387] = np.asarray(gateR_b).reshape(())
    pkf[C, PK_BLROW] = np.asarray(concaL_b).reshape(C)
    pkf[C, PK_BRROW] = np.asarray(concaR_b).reshape(C)
    pkf[0:C, PK_BRCOL] = np.asarray(concaR_b).reshape(C, 1)
    pkf[:, PK_ONECOL] = 1.0
    pkf[0:C, PK_ID64] = np.eye(C, dtype=f)

    pkb = np.zeros((C2, PKB_COLS), dtype=np.float32)
    pkb[:, 0:128] = np.eye(C2, dtype=f)
    pkb[0, 128:192] = 1.0
    pkb[C, 128:192] = 1.0
    import jax.numpy as jnp
    pkb16 = np.asarray(jnp.asarray(pkb, dtype=jnp.bfloat16))

    return {"pkf": np.ascontiguousarray(pkf),
            "pkb": np.ascontiguousarray(pkb16)}


def kernel(xlh, xll, xrh, xrl,
           concaL_w, concaL_b, concaR_w, concaR_b,
           gateL_w, gateL_b, gateR_w, gateR_b,
           concaLo_w, concaLo_b, concaRo_w, concaRo_b,
           _return_results=False):
    nc = _get_nc()
    shared = _prep_shared(concaL_w, concaL_b, concaR_w, concaR_b,
                          gateL_w, gateL_b, gateR_w, gateR_b,
                          concaLo_w, concaLo_b, concaRo_w, concaRo_b)
    xlh = np.asarray(xlh, dtype=np.float32)
    xll = np.asarray(xll, dtype=np.float32)
    xrh = np.asarray(xrh, dtype=np.float32)
    xrl = np.asarray(xrl, dtype=np.float32)

    in_maps = []
    for c in range(B):
        x2l = np.concatenate([xlh[c].reshape(C, HW), xll[c].reshape(C, HW)], axis=0)
        x2r = np.concatenate([xrh[c].reshape(C, HW), xrl[c].reshape(C, HW)], axis=0)
        m = dict(shared)
        m["x2l"] = np.ascontiguousarray(x2l)
        m["x2r"] = np.ascontiguousarray(x2r)
        in_maps.append(m)

    # The first execution of a freshly compiled NEFF occasionally hits a
    # transient NRT_EXEC_UNIT_UNRECOVERABLE on this axon setup; an immediate
    # re-dispatch of the same executable has always succeeded, so retry.
    res = None
    for attempt in range(3):
        try:
            res = run_bass_kernel_spmd(nc, in_maps, list(range(B)))
            break
        except Exception:
            if attempt == 2:
                raise
            import time as _time
            _time.sleep(2.0)
    out_L = np.stack([res.results[c]["out_l"].reshape(C, H, W) for c in range(B)])
    out_R = np.stack([res.results[c]["out_r"].reshape(C, H, W) for c in range(B)])
    if _return_results:
        return (out_L, out_R), res
    return (out_L, out_R)


# revision 20
# speedup vs baseline: 1.1600x; 1.0326x over previous
"""CoAttenBlock Trainium2 kernel.

Full inputs in, full outputs out. Shards batch (B=8) across 8 NeuronCores,
one sample per core (pure data parallel, no collectives).

Per-core math (C=64, HW=2304, strips of 128 along the left position axis n):
  XL = WL @ [xlh;xll] + bL                      [64, 2304]
  XR = WR @ [xrh;xrl] + bR
  per strip s: aff_s = XL[:,s].T @ XR           [128, 2304]
               E_s   = exp(aff_s) -> bf16, rowsum via activation accum_out
               r2    = 1/rowsum  (folded into the strip's YRT weight columns)
               P12  += [YLT_s | YRT_s*r2].T @ E_s   (PSUM-resident [128, 2304])
  where YLT = (WLo_r @ XL).T strips, YRT = (WRo_r @ XR).T strips, so that
  P1 = WLo_r @ (XL @ E) and P2 = WRo_r @ (XR @ diag(r2) @ E).
  Gate pre-acts are recovered as vL.T @ P1 with vL = solve(WLo_r.T, gwL.T)
  (host-side 64x64 solve; inputs are deterministic, conditioning ~5e2).
  E/Wc/colsum run in bf16: bacc matmuls stay 1 cycle/row and the single
  colsum accumulator gets DVE's 2x two-byte mode; the ~0.5% bf16
  accumulation error only touches the sigmoid gates (budget 2e-2).
  All affs of a strip are emitted before the lagged bacc matmuls so a bacc
  stall (waiting on the r2 scale) can never block the aff->exp stream on
  the in-order PE queue.
  Tail gate math runs in a transposed [128,18] layout (column m = 128*mt+mp
  lives at partition mp, col mt) so ACT/DVE ops are 18 cols wide, then a
  bf16 PE transpose + SBUF->SBUF DMA rebuilds the row form for the
  ones-broadcast matmuls:
  s1 = sigmoid(g1pre*r1 + gb1)*r1, r1 = 1/colsum ; s2 = sigmoid(g2pre+gb2)
  out_L = WLo_l @ XLa + P1*s1[m] ; out_R = WRo_l @ XRa + P2*s2[m]
  (bLo/bRo folded in via an augmented ones row 64 on XLa/XRa).

float32r (single-pass PE mode) is used for the aff/conv/out matmuls;
producers of f32r-matmul inputs write with a f32r-typed output AP so the
engine rounds on write (BIR verifier requirement).
"""

import os
import sys

import numpy as np

if os.path.isdir("/opt/trn_rl_repo") and "/opt/trn_rl_repo" not in sys.path:
    sys.path.insert(0, "/opt/trn_rl_repo")

import concourse.bass as bass
import concourse.tile as tile
from concourse import bacc, mybir
from concourse.bass_utils import run_bass_kernel_spmd

B, C, H, W = 8, 64, 48, 48
HW = H * W            # 2304
C2 = 2 * C            # 128
NSTRIP = HW // 128    # 18
F32 = mybir.dt.float32
F32R = mybir.dt.float32r
BF16 = mybir.dt.bfloat16
AF = mybir.ActivationFunctionType


def chunks(total, step=512):
    out = []
    c0 = 0
    while c0 < total:
        out.append((c0, min(step, total - c0)))
        c0 += step
    return out


CH_2304 = chunks(2304)            # 4x512 + 256

# f32 weight-pack column layout
PK_WLT = slice(0, 64)
PK_WRT = slice(64, 128)
PK_WLORT = slice(128, 192)
PK_WRORT = slice(192, 256)
PK_WLOLTA = slice(256, 320)
PK_WROLTA = slice(320, 384)
PK_VLR2 = slice(384, 386)
PK_GBL = slice(386, 387)
PK_GBR = slice(387, 388)
PK_BLROW = slice(388, 452)
PK_BRCOL = slice(452, 453)
PKF_COLS = 453
# bf16 pack: id128 [0:128], ones64 rows0/64 [128:192], ones col [192], id64 [196:260]
PKB_COLS = 260


def r(ap):
    return ap.bitcast(F32R)


def build_nc():
    nc = bacc.Bacc("TRN2", target_bir_lowering=False, debug=False)

    x2l_d = nc.dram_tensor("x2l", [C2, HW], F32, kind="ExternalInput").ap()
    x2r_d = nc.dram_tensor("x2r", [C2, HW], F32, kind="ExternalInput").ap()
    pkf_d = nc.dram_tensor("pkf", [C2, PKF_COLS], F32, kind="ExternalInput").ap()
    pkb_d = nc.dram_tensor("pkb", [C2, PKB_COLS], BF16, kind="ExternalInput").ap()
    ones_np = np.ones((1, HW), dtype=np.float32)
    ones_d = nc.inline_tensor(ones_np, "onesrow").ap()

    out_l_d = nc.dram_tensor("out_l", [C, HW], F32, kind="ExternalOutput").ap()
    out_r_d = nc.dram_tensor("out_r", [C, HW], F32, kind="ExternalOutput").ap()

    with tile.TileContext(nc) as tc:
        import contextlib

        with contextlib.ExitStack() as outer:
            consts = outer.enter_context(tc.tile_pool(name="consts", bufs=1))
            big = outer.enter_context(tc.tile_pool(name="big", bufs=1))
            epool = outer.enter_context(tc.tile_pool(name="epool", bufs=6))
            smalls = outer.enter_context(tc.tile_pool(name="smalls", bufs=3))
            tailsb = outer.enter_context(tc.tile_pool(name="tailsb", bufs=1))

            pk = consts.tile([C2, PKF_COLS], F32)
            pkb = consts.tile([C2, PKB_COLS], BF16)

            # ---- big SBUF tensors ----
            x2l = big.tile([C2, HW], F32)
            x2r = big.tile([C2, HW], F32)
            XLa = big.tile([C + 1, HW], F32)   # row 64 = ones (bias fold)
            XRa = big.tile([C + 1, HW], F32)
            Wc = big.tile([C2, HW], BF16)      # 18 strips of [YLT | YRT]
            csum = big.tile([C2, HW], BF16)    # colsum accumulator (DVE 2x)
            P12sb = big.tile([C2, HW], F32)    # drained P1 (0:64) / P2 (64:128)
            outLR = big.tile([C2, HW], F32)

            # Input DMA order tuned for the phase-1 pipeline: weights, then
            # chunks in first-use order. ones rows + bf16 pack go through the
            # ACT-issued queue in parallel with the SP queue.
            LCH = [(0, 768), (768, 768), (1536, 768)]
            RCH = [(0, 768), (768, 768), (1536, 768)]
            nc.sync.dma_start(out=r(pk), in_=r(pkf_d))
            nc.scalar.dma_start(out=r(XLa[C:C + 1, :]), in_=r(ones_d))
            nc.scalar.dma_start(out=r(XRa[C:C + 1, :]), in_=r(ones_d))
            nc.scalar.dma_start(out=pkb, in_=pkb_d)

            def dma_in(dst, src, c0, cn):
                nc.sync.dma_start(out=r(dst[:, c0:c0 + cn]),
                                  in_=r(src[:, c0:c0 + cn]))

            dma_in(x2l, x2l_d, *LCH[0])
            dma_in(x2r, x2r_d, *RCH[0])
            dma_in(x2r, x2r_d, *RCH[1])
            dma_in(x2l, x2l_d, *LCH[1])
            dma_in(x2r, x2r_d, *RCH[2])
            dma_in(x2l, x2l_d, *LCH[2])

            wlT = r(pk[:, PK_WLT])
            wrT = r(pk[:, PK_WRT])
            wloRT = r(pk[0:C, PK_WLORT])
            wroRT = r(pk[0:C, PK_WRORT])
            wloLTa = r(pk[0:C + 1, PK_WLOLTA])
            wroLTa = r(pk[0:C + 1, PK_WROLTA])
            vlr2f = pk[:, PK_VLR2]
            gbL = pk[:, PK_GBL]
            gbR = pk[:, PK_GBR]
            bLrow = r(pk[C:C + 1, PK_BLROW])
            bRcol = pk[0:C, PK_BRCOL]
            id128b = pkb[:, 0:128]
            ones64b = pkb[:, 128:192]
            onecolb = pkb[:, 192:193]
            id64b = pkb[0:C, 196:260]

            with contextlib.ExitStack() as ph2_psum:
                p12p = ph2_psum.enter_context(
                    tc.tile_pool(name="p12p", bufs=1, space="PSUM"))
                affp = ph2_psum.enter_context(
                    tc.tile_pool(name="affp", bufs=1, space="PSUM"))
                P12 = p12p.tile([C2, HW], F32)  # lives all of phase 1+2
                ring = affp.tile([C2, 1536], F32, tag="ring", name="aff_ring")

                # ---- phase 1 pieces: conv scratch lives inside P12
                # (rows 0:64 for L, 64:128 for R; bacc starts 2 strips in) ----
                def conv_L(j):
                    c0, cn = CH_2304[j]
                    sl = slice(c0, c0 + cn)
                    nc.tensor.matmul(P12[0:C, sl], wlT, r(x2l[:, sl]),
                                     start=True, stop=False)
                    nc.tensor.matmul(P12[0:C, sl], bLrow, r(XLa[C:C + 1, sl]),
                                     start=False, stop=True)
                    nc.vector.tensor_copy(r(XLa[0:C, sl]), P12[0:C, sl])

                def conv_R(j):
                    c0, cn = CH_2304[j]
                    sl = slice(c0, c0 + cn)
                    rsl = (j % 3) * 512
                    nc.tensor.matmul(ring[0:C, rsl:rsl + cn], wrT, r(x2r[:, sl]),
                                     start=True, stop=True)
                    nc.scalar.activation(r(XRa[0:C, sl]), ring[0:C, rsl:rsl + cn],
                                         AF.Identity, bias=bRcol, scale=1.0)

                def emit_y(t, drain_eng):
                    # Wc strip t = [YLT_t | YRT_t]; PSUM scratch inside P12
                    # (pre-bacc window only: strips 0-1 of the loop).
                    nc.tensor.matmul(P12[:, 128 * t:128 * t + 64],
                                     r(XLa[0:C, 128 * t:128 * t + 128]),
                                     wloRT, start=True, stop=True)
                    nc.tensor.matmul(P12[:, 128 * t + 64:128 * t + 128],
                                     r(XRa[0:C, 128 * t:128 * t + 128]),
                                     wroRT, start=True, stop=True)
                    dst = Wc[:, 128 * t:128 * t + 128]
                    srcp = P12[:, 128 * t:128 * t + 128]
                    if drain_eng == "dve":
                        nc.vector.tensor_copy(dst, srcp)
                    else:
                        nc.scalar.copy(dst, srcp)

                phase = 0
                r2s = {}

                def emit_bacc(sb, c0, cn):
                    nc.tensor.matmul(P12[:, c0:c0 + cn],
                                     Wc[:, 128 * sb:128 * sb + 128],
                                     Es[sb][:, c0:c0 + cn],
                                     start=(sb == 0), stop=(sb == NSTRIP - 1))

                def emit_csum(sb):
                    if sb == 0:
                        nc.vector.tensor_copy(csum, Es[sb])
                    else:
                        nc.vector.tensor_add(csum, csum, Es[sb])

                # schedules for interleaving phase-1 work into strips 0/1:
                # section i of strip 0/1 emits these conv/emit_y pieces.
                L_SCHED = {0: {0: [0], 3: [1], 4: [2]}, 1: {0: [3], 1: [4]}}
                R_SCHED = {0: {0: [0], 1: [1], 2: [2], 3: [3], 4: [4]}, 1: {}}

                def emit_A(s, phase, sb):
                    # scale strip sb's YRT by r2, affs+exps for strip s, then
                    # the lagged bacc matmuls for sb (after ALL affs so a bacc
                    # stall can't block the exp stream on in-order PE).
                    if sb >= 0:
                        wright = Wc[:, 128 * sb + 64:128 * sb + 128]
                        nc.vector.tensor_scalar_mul(wright, wright, r2s[sb])
                    E = epool.tile([C2, HW], BF16, tag="e", name=f"E_{s}")
                    rs = smalls.tile([C2, 4], F32, tag="rs", name=f"rs_{s}")
                    lhs_aff = r(XLa[0:C, 128 * s:128 * s + 128])
                    pieces = [(p0, pn, (phase + i) % 3)
                              for i, (p0, pn) in enumerate(CH_2304)]
                    groups = []
                    for p0, pn, sl in pieces:
                        if groups and groups[-1][2] + groups[-1][1] == sl * 512 \
                                and groups[-1][1] + pn <= 1536:
                            groups[-1][1] += pn
                        else:
                            groups.append([p0, pn, sl * 512])
                    gidx = 0
                    done = 0
                    for i, (p0, pn, sl) in enumerate(pieces):
                        for j in L_SCHED.get(s, {}).get(i, []):
                            conv_L(j)
                        for j in R_SCHED.get(s, {}).get(i, []):
                            conv_R(j)
                        nc.tensor.matmul(ring[:, sl * 512:sl * 512 + pn],
                                         lhs_aff, r(XRa[0:C, p0:p0 + pn]),
                                         start=True, stop=True)
                        done += pn
                        while gidx < len(groups) and \
                                groups[gidx][0] + groups[gidx][1] <= done:
                            m0, mn, r0 = groups[gidx]
                            nc.scalar.activation(E[:, m0:m0 + mn],
                                                 ring[:, r0:r0 + mn], AF.Exp,
                                                 accum_out=rs[:, gidx:gidx + 1])
                            gidx += 1
                        if s == 0 and i >= 1:
                            for t in (2 * (i - 1), 2 * (i - 1) + 1):
                                emit_y(t, "dve")
                        if s == 1:
                            for t in (8 + 2 * i, 8 + 2 * i + 1):
                                if t < NSTRIP:
                                    emit_y(t, "dve")
                    if sb >= 0:
                        for bc0, bcn in CH_2304:
                            emit_bacc(sb, bc0, bcn)
                    rowsum = smalls.tile([C2, 1], F32, tag="rowsum",
                                         name=f"rowsum_{s}")
                    r2 = smalls.tile([C2, 1], F32, tag="r2", name=f"r2_{s}",
                                     bufs=4)
                    nc.vector.tensor_reduce(rowsum, rs[:, 0:len(groups)],
                                            axis=mybir.AxisListType.X,
                                            op=mybir.AluOpType.add)
                    nc.vector.reciprocal(r2, rowsum)
                    r2s[s] = r2
                    if sb >= 0:
                        emit_csum(sb)
                    return E

                def emit_B_tail(sb):
                    wright = Wc[:, 128 * sb + 64:128 * sb + 128]
                    nc.vector.tensor_scalar_mul(wright, wright, r2s[sb])
                    # csum first: the tail gate chain waits on the full colsum,
                    # so get it onto DVE before the P12 drains queue up.
                    emit_csum(sb)
                    drain_eng = ["dve", "act", "dve", "act", "dve"]
                    for j, (c0, cn) in enumerate(CH_2304):
                        emit_bacc(sb, c0, cn)
                        if sb == NSTRIP - 1:
                            # P12 piece is final once the last strip's bacc
                            # for it retires: drain immediately.
                            sl = slice(c0, c0 + cn)
                            if drain_eng[j] == "act":
                                nc.scalar.copy(r(P12sb[:, sl]), P12[:, sl])
                            else:
                                nc.vector.tensor_copy(r(P12sb[:, sl]), P12[:, sl])

                Es = {}
                for s in range(NSTRIP):
                    Es[s] = emit_A(s, phase, s - 2)
                    phase = (phase + len(CH_2304)) % 3
                for s in (NSTRIP - 2, NSTRIP - 1):
                    emit_B_tail(s)

            # ---- phase 3: transposed gate chain + broadcast/gate/out ----
            with tc.tile_pool(name="ph3p", bufs=1, space="PSUM") as ph3:
                # one shared bank for all the small tail tiles
                tailps = ph3.tile([C2, 512], F32)
                cst = tailps[:, 0:NSTRIP]
                gpt = tailps[:, 32:32 + 2 * NSTRIP]
                s12T = tailps[0:2 * NSTRIP, 128:192].bitcast(BF16)  # [36,128]
                # colsum transposed: cst[mp, mt] = sum_p csum[p, 128*mt+mp]
                for mt in range(NSTRIP):
                    m0 = 128 * mt
                    nc.tensor.matmul(cst[:, mt:mt + 1],
                                     csum[:, m0:m0 + 128], onecolb,
                                     start=True, stop=True)
                    # gate pre-acts transposed, both sides at once
                    nc.tensor.matmul(gpt[:, 2 * mt:2 * mt + 2],
                                     P12sb[:, m0:m0 + 128], vlr2f,
                                     start=True, stop=True)
                r1t = tailsb.tile([C2, NSTRIP], F32)
                nc.vector.reciprocal(r1t, cst)
                g1pre = tailsb.tile([C2, NSTRIP], F32)
                nc.vector.tensor_mul(g1pre, gpt[:, 0::2], r1t)
                g1t = tailsb.tile([C2, NSTRIP], F32)
                s12j = tailsb.tile([C2, 2 * NSTRIP], BF16)
                nc.scalar.activation(g1t, g1pre, AF.Sigmoid, bias=gbL, scale=1.0)
                nc.scalar.activation(s12j[:, NSTRIP:], gpt[:, 1::2], AF.Sigmoid,
                                     bias=gbR, scale=1.0)
                nc.vector.tensor_mul(s12j[:, 0:NSTRIP], g1t, r1t)
                # transpose to rows, rebuild [rows 0/64, 2304] layout via two
                # SBUF->SBUF DMAs on separate issue queues
                nc.tensor.transpose(s12T, s12j, id128b)
                s12Ts = tailsb.tile([2 * NSTRIP, 128], BF16)
                nc.vector.tensor_copy(s12Ts, s12T)
                srow = tailsb.tile([C + 1, HW], BF16)  # s1 row 0, s2 row 64
                nc.sync.dma_start(out=srow[0:1, :], in_=s12Ts[0:NSTRIP, :])
                nc.scalar.dma_start(out=srow[C:C + 1, :], in_=s12Ts[NSTRIP:, :])

                # per-chunk: S broadcast, gate-mul, out conv + id-add, drain
                for q, (p0, pn) in enumerate(CH_2304):
                    sl = slice(p0, p0 + pn)
                    for side in (0, 1):
                        rows = slice(0, C) if side == 0 else slice(C, C2)
                        S = ph3.tile([C, pn], F32, tag=f"S{side}",
                                     name=f"S{side}_{q}", padded_shape=[C, 512],
                                     bufs=2)
                        srow_r = srow[0:1, sl] if side == 0 else srow[C:C + 1, sl]
                        ones_r = ones64b[0:1, :] if side == 0 else ones64b[C:C + 1, :]
                        nc.tensor.matmul(S, ones_r, srow_r,
                                         start=True, stop=True)
                        t_ = tailsb.tile([C, pn], BF16, tag=f"t{side}",
                                         name=f"t{side}_{q}",
                                         padded_shape=[C, 512], bufs=2)
                        nc.vector.tensor_mul(t_, P12sb[rows, sl], S)
                        O = ph3.tile([C, pn], F32, tag=f"O{side}",
                                     name=f"O{side}_{q}", padded_shape=[C, 512])
                        Xa = XLa if side == 0 else XRa
                        wA = wloLTa if side == 0 else wroLTa
                        nc.tensor.matmul(O, wA, r(Xa[:, sl]),
                                         start=True, stop=False)
                        nc.tensor.matmul(O, id64b, t_,
                                         start=False, stop=True)
                        nc.scalar.copy(r(outLR[rows, sl]), O)
                    if p0 + pn in (1024, 2048):
                        d0 = p0 + pn - 1024
                        nc.sync.dma_start(out=out_l_d[:, d0:p0 + pn],
                                          in_=outLR[0:C, d0:p0 + pn])
                        nc.sync.dma_start(out=out_r_d[:, d0:p0 + pn],
                                          in_=outLR[C:C2, d0:p0 + pn])
                nc.sync.dma_start(out=out_l_d[:, 2048:], in_=outLR[0:C, 2048:])
                nc.sync.dma_start(out=out_r_d[:, 2048:], in_=outLR[C:C2, 2048:])

    nc.compile()
    return nc


_NC_CACHE = {}


def _get_nc():
    if "nc" not in _NC_CACHE:
        _NC_CACHE["nc"] = build_nc()
    return _NC_CACHE["nc"]


def _prep_shared(concaL_w, concaL_b, concaR_w, concaR_b,
                 gateL_w, gateL_b, gateR_w, gateR_b,
                 concaLo_w, concaLo_b, concaRo_w, concaRo_b):
    f = np.float32
    wloR = np.asarray(concaLo_w)[:, C:].astype(np.float64)
    wroR = np.asarray(concaRo_w)[:, C:].astype(np.float64)
    vL = np.linalg.solve(wloR.T, np.asarray(gateL_w).astype(np.float64).reshape(C))
    vR = np.linalg.solve(wroR.T, np.asarray(gateR_w).astype(np.float64).reshape(C))

    pkf = np.zeros((C2, PKF_COLS), dtype=f)
    pkf[:, PK_WLT] = np.asarray(concaL_w).T
    pkf[:, PK_WRT] = np.asarray(concaR_w).T
    pkf[0:C, PK_WLORT] = wloR.T
    pkf[0:C, PK_WRORT] = wroR.T
    pkf[0:C, PK_WLOLTA] = np.asarray(concaLo_w)[:, :C].T
    pkf[C, PK_WLOLTA] = np.asarray(concaLo_b).reshape(C)
    pkf[0:C, PK_WROLTA] = np.asarray(concaRo_w)[:, :C].T
    pkf[C, PK_WROLTA] = np.asarray(concaRo_b).reshape(C)
    pkf[0:C, 384] = vL
    pkf[C:C2, 385] = vR
    pkf[:, 386] = np.asarray(gateL_b).reshape(())
    pkf[:, 387] = np.asarray(gateR_b).reshape(())
    pkf[C, PK_BLROW] = np.asarray(concaL_b).reshape(C)
    pkf[0:C, PK_BRCOL] = np.asarray(concaR_b).reshape(C, 1)

    pkb = np.zeros((C2, PKB_COLS), dtype=np.float32)
    pkb[:, 0:128] = np.eye(C2, dtype=f)
    pkb[0, 128:192] = 1.0
    pkb[C, 128:192] = 1.0
    pkb[:, 192] = 1.0
    pkb[0:C, 196:260] = np.eye(C, dtype=f)
    import jax.numpy as jnp
    pkb16 = np.asarray(jnp.asarray(pkb, dtype=jnp.bfloat16))

    return {"pkf": np.ascontiguousarray(pkf),
            "pkb": np.ascontiguousarray(pkb16)}


def kernel(xlh, xll, xrh, xrl,
           concaL_w, concaL_b, concaR_w, concaR_b,
           gateL_w, gateL_b, gateR_w, gateR_b,
           concaLo_w, concaLo_b, concaRo_w, concaRo_b,
           _return_results=False):
    nc = _get_nc()
    shared = _prep_shared(concaL_w, concaL_b, concaR_w, concaR_b,
                          gateL_w, gateL_b, gateR_w, gateR_b,
                          concaLo_w, concaLo_b, concaRo_w, concaRo_b)
    xlh = np.asarray(xlh, dtype=np.float32)
    xll = np.asarray(xll, dtype=np.float32)
    xrh = np.asarray(xrh, dtype=np.float32)
    xrl = np.asarray(xrl, dtype=np.float32)

    in_maps = []
    for c in range(B):
        x2l = np.concatenate([xlh[c].reshape(C, HW), xll[c].reshape(C, HW)], axis=0)
        x2r = np.concatenate([xrh[c].reshape(C, HW), xrl[c].reshape(C, HW)], axis=0)
        m = dict(shared)
        m["x2l"] = np.ascontiguousarray(x2l)
        m["x2r"] = np.ascontiguousarray(x2r)
        in_maps.append(m)

    # The first execution of a freshly compiled NEFF occasionally hits a
    # transient NRT_EXEC_UNIT_UNRECOVERABLE on this axon setup; an immediate
    # re-dispatch of the same executable has always succeeded, so retry.
    res = None
    for attempt in range(3):
        try:
            res = run_bass_kernel_spmd(nc, in_maps, list(range(B)))
            break
        except Exception:
            if attempt == 2:
                raise
            import time as _time
            _time.sleep(2.0)
    out_L = np.stack([res.results[c]["out_l"].reshape(C, H, W) for c in range(B)])
    out_R = np.stack([res.results[c]["out_r"].reshape(C, H, W) for c in range(B)])
    if _return_results:
        return (out_L, out_R), res
    return (out_L, out_R)
